# revision 87
# baseline (speedup 1.0000x reference)
"""BinaryLinear on 8 TRN2 NeuronCores.

Computes y = sign(x) @ sign(W)^T + bias for x [8192, 4096] fp32,
W [4096, 4096] fp32, bias [4096] fp32 -> y [8192, 4096] fp32.

Sharding: 4 batch shards x 2 out-feature shards (one block per core).
Each core computes the transposed block out^T [2048 o, 2048 b] in fp16
(sums are exact integers; fp16 rounds once with the bias), and the host
transposes blocks back to fp32 while assembling the full output.

The kernel is DMA-bound: 67.1 MB of fp32 inputs + 8.4 MB of fp16
output per core over the shared ~360 GB/s DMA bus ~= 210 us. Design:
  - All input loads stream back-to-back on the SP HWDGE ring in one
    planned order (x chunk 0, W0-1, chunk 1, W2-5, chunks 2-3, W6-15);
    output stores ride the otherwise-idle Pool SWDGE ring so they never
    block the Sign/prep pipeline.
  - x loads in [128, 2048] halves; W in [128, 1024] quarter-spans.
    Per span: ACT Sign (fp32 -> fp8, +-1 exact), PE transposes of the
    fp8 bytes viewed as u16 pairs, DVE copy into the resident
    DoubleRowSwInterleave operands (wT reversed-column per block).
  - Matmul emission lags W prep by MM_LAG spans and is pumped BEFORE
    each prep, so in-order engine queues never park prep work behind
    matmuls (PE) or biases behind copies (DVE).
  - While a batch chunk is still loading, units of already-prepped W
    tiles open early in "sliced" mode (one 128-column batch sub-tile
    per matmul) so PE has work during every chunk load; open sliced
    units are capped by the PSUM pool size to stay deadlock-free.
  - PSUM: 6 rotating accumulation banks + 2 shared transpose banks;
    deep osb/staging pools keep loads ahead of compute recycling.
"""

from contextlib import ExitStack

import numpy as np

import concourse.bacc as bacc
import concourse.bass as bass
import concourse.mybir as mybir
import concourse.tile as tile
from concourse import masks
from concourse.bass_utils import run_bass_kernel_spmd

F32 = mybir.dt.float32
F16 = mybir.dt.float16
BF16 = mybir.dt.bfloat16
AF = mybir.ActivationFunctionType
ALU = mybir.AluOpType

B, IN, OUT = 8192, 4096, 4096
P_B, P_O = 4, 2                  # batch shards x out-feature shards
Bs, Os = B // P_B, OUT // P_O    # per-core shard sizes
N_CORES = P_B * P_O

MM_LAG = 2                       # W spans prepped ahead of matmul emission


def _build(Bs=2048, Os=2048, K=4096, mm_dtype=mybir.dt.float8e4):
    nc = bacc.Bacc("TRN2", target_bir_lowering=False, debug=False)

    x = nc.declare_dram_parameter("x", [Bs, K], F32, isOutput=False)
    w = nc.declare_dram_parameter("w", [Os, K], F32, isOutput=False)
    b = nc.declare_dram_parameter("b", [Os], F32, isOutput=False)
    out = nc.declare_dram_parameter("out", [Os, Bs], F16, isOutput=True)

    T = K // 256       # DoubleRow pair-groups
    OT = Os // 128     # W row tiles (partition dim of out^T)
    KH = K // 2        # fp32 columns per x half-tile
    KQ = K // 4        # fp32 columns per W quarter-span

    def w_spans(ot):
        """k-spans per W tile (quarter granularity)."""
        return [(k0, k0 + KQ) for k0 in range(0, K, KQ)]

    # batch chunks (b0, width)
    CHUNKS = [(0, 512), (512, 512), (1024, 512), (1536, 512)]
    NCH = len(CHUNKS)

    with tile.TileContext(nc) as tc, ExitStack() as ctx:
        const = ctx.enter_context(tc.tile_pool(name="const", bufs=1))
        wt_pool = ctx.enter_context(tc.tile_pool(name="wt", bufs=1))
        xt_pool = ctx.enter_context(tc.tile_pool(name="xt", bufs=NCH))
        xstage = ctx.enter_context(tc.tile_pool(name="xstage", bufs=4))
        wstage = ctx.enter_context(tc.tile_pool(name="wstage", bufs=6))
        sx_pool = ctx.enter_context(tc.tile_pool(name="sx", bufs=2))
        sw_pool = ctx.enter_context(tc.tile_pool(name="sw", bufs=4))
        ptr_pool = ctx.enter_context(
            tc.tile_pool(name="ptr", bufs=2, space=bass.MemorySpace.PSUM)
        )
        pacc_pool = ctx.enter_context(
            tc.tile_pool(name="pacc", bufs=6, space=bass.MemorySpace.PSUM)
        )
        outsb = ctx.enter_context(tc.tile_pool(name="outsb", bufs=8))

        ident16 = const.tile([128, 128], BF16)
        masks.make_identity(nc, ident16[:])
        identf = const.tile([OT, OT], F32)
        masks.make_identity(nc, identf[:])

        # bias: one contiguous [OT, 128] load (16 descriptors, no 4-byte
        # element penalty), then a tiny PE transpose to [128, OT]
        bias_raw = const.tile([OT, 128], F32)
        nc.scalar.dma_start(bias_raw[:], b.rearrange("(ot p) -> ot p", p=128))
        bias_sb = const.tile([128, OT], F32)
        pbias = ptr_pool.tile([128, OT], F32, tag="ptr", name="pbias")
        nc.tensor.transpose(pbias[:], bias_raw[:], identf[:])
        nc.vector.tensor_copy(bias_sb[:], pbias[:])

        # wT u16-pair view [128, T, Os]: partition p of group t holds the
        # fp8 byte pair (k = 256t+2p, 256t+2p+1); m reversed per ot block.
        wT = wt_pool.tile([128, T, Os], BF16)
        # xTp[c] [128, T, width]: same pair layout per batch column; all
        # chunk tiles stay live in rotating buffers of their pool.
        xTp = [xt_pool.tile([128, T, wc], BF16, name="xTp", tag="xTp")
               for (_, wc) in CHUNKS]

        def prep_x_half(c, s, h):
            """Load 128 x rows (sub-tile s of chunk c), cols [h*KH, +KH);
            sign -> u16-pair PE transposes -> DVE copy into xTp[c]."""
            b0, wc = CHUNKS[c]
            xs = xstage.tile([128, KH], F32, tag="xs", name="xs")
            r0 = b0 + s * 128
            nc.sync.dma_start(xs[:], x[r0:r0 + 128, h * KH:(h + 1) * KH])
            sx = sx_pool.tile([128, KH], mm_dtype, tag="sx", name="sx")
            nc.scalar.activation(sx[:], xs[:], AF.Sign)
            sx16 = sx.bitcast(BF16)   # [128, KH//2]
            nblk = KH // 256          # 128-col bf16 blocks = pair groups
            ptx = ptr_pool.tile([128, nblk * 128], BF16, tag="ptr",
                                name="ptrx")
            for j in range(nblk):
                nc.tensor.transpose(
                    ptx[:, j * 128:(j + 1) * 128],
                    sx16[:, j * 128:(j + 1) * 128],
                    ident16[:],
                )
            t0 = h * nblk
            nc.vector.tensor_copy(
                xTp[c][:, t0:t0 + nblk, s * 128:(s + 1) * 128], ptx[:]
            )

        def prep_w_span(ot, si):
            """Load W rows [ot*128, +128) cols of span si; sign ->
            u16-pair PE transposes -> reversed-m DVE copy (SwInterleave)."""
            k0, k1 = w_spans(ot)[si]
            cols = k1 - k0
            ws = wstage.tile([128, cols], F32, tag="ws", name="ws")
            nc.sync.dma_start(ws[:], w[ot * 128:(ot + 1) * 128, k0:k1])
            sw = sw_pool.tile([128, cols], mm_dtype, tag="sw", name="sw")
            nc.scalar.activation(sw[:], ws[:], AF.Sign)
            sw16 = sw.bitcast(BF16)   # [128, cols//2]
            nblk = cols // 256
            ptx = ptr_pool.tile([128, nblk * 128], BF16, tag="ptr",
                                name="ptxw")
            for j in range(nblk):
                nc.tensor.transpose(
                    ptx[:, j * 128:(j + 1) * 128],
                    sw16[:, j * 128:(j + 1) * 128],
                    ident16[:],
                )
            t0 = k0 // 256
            nc.vector.tensor_copy(
                wT[:, t0:t0 + nblk, ot * 128:(ot + 1) * 128][:, :, ::-1],
                ptx[:],
            )

        def mm_span(ot, c, pacc, si, s=None):
            """DoubleRow matmuls for pair-groups of span si into pacc.
            With s, restrict to batch sub-tile s (128 columns)."""
            _, wc = CHUNKS[c]
            xTp8 = xTp[c].bitcast(mm_dtype)   # [128, T, 2*wc]
            k0, k1 = w_spans(ot)[si]
            t0, t1 = k0 // 256, k1 // 256
            b0, b1 = (0, wc) if s is None else (s * 128, (s + 1) * 128)
            for t in range(t0, t1):
                rhs = xTp8[:, t, 2 * b0:2 * b1].rearrange(
                    "p (b h) -> p h b", h=2
                )
                nc.tensor.matmul(
                    pacc[:, b0:b1],
                    wT[:, t, ot * 128:(ot + 1) * 128].bitcast(mm_dtype),
                    rhs,
                    start=(t == 0),
                    stop=(t == T - 1),
                    perf_mode=mybir.MatmulPerfMode.DoubleRowSwInterleave,
                )

        def finish_unit(ot, c, pacc):
            b0, wc = CHUNKS[c]
            osb = outsb.tile([128, wc], F16, name="osb", tag="osb")
            nc.vector.tensor_scalar(
                osb[:], pacc[:, :wc], bias_sb[:, ot:ot + 1], None, ALU.add
            )
            # the last tile's stores ride the SP ring (idle once loads are
            # done; HWDGE issue is ~0.4us shorter than Pool SWDGE desc-gen,
            # which sits on the final store's critical chain)
            eng = nc.sync if ot == OT - 1 else nc.gpsimd
            eng.dma_start(
                out[ot * 128:(ot + 1) * 128, b0:b0 + wc], osb[:]
            )

        # ---- planned bus order for the SP load stream ----
        def chunk_items(c):
            _, wc = CHUNKS[c]
            return [("x", c, s, h) for s in range(wc // 128) for h in (0, 1)]

        def wtile_items(ot):
            return [("w", ot, si) for si in range(len(w_spans(ot)))]

        order = []
        order += chunk_items(0)
        order += wtile_items(0) + wtile_items(1)
        order += chunk_items(1)
        for ot in range(2, 6):
            order += wtile_items(ot)
        order += chunk_items(2)
        for c in range(3, NCH):
            order += chunk_items(c)
        for ot in range(6, OT):
            order += wtile_items(ot)

        # W-prep sequence index of each span, in bus order
        wseq = {}
        g = 0
        for item in order:
            if item[0] == "w":
                wseq[(item[1], item[2])] = g
                g += 1

        # ---- emission: preps in bus order; matmuls lag W prep by MM_LAG
        # spans and are pumped before each prep. Units of a still-loading
        # chunk may open early in sliced mode (per 128-col sub-tile) so PE
        # has work during chunk loads; at most EARLY_CAP such units stay
        # open (each holds a pacc bank until its chunk completes). ----
        EARLY_CAP = 6
        x_done = {}          # c -> count of prepped halves
        xh_ok = set()        # (c, s, h) prepped
        chunk_ok = set()     # chunks fully prepped
        prepped_g = -1       # highest prepped W span gid (contiguous)
        unit_si = {}         # (ot, c) -> next span index (whole mode)
        slice_si = {}        # (ot, c, s) -> next span index (sliced mode)
        sliced = set()       # units opened in sliced mode
        unit_pacc = {}       # (ot, c) -> open pacc tile
        emitted = set()      # finished units

        def subtiles(c):
            return CHUNKS[c][1] // 128

        def pump(lag=MM_LAG, final=False):
            limit = prepped_g if final else prepped_g - lag

            def ok(ot, si):
                return wseq[(ot, si)] <= limit

            n_early = len([u for u in unit_pacc if u in sliced])
            for ot in range(OT):
                if not ok(ot, 0):
                    continue
                nsp = len(w_spans(ot))
                for c in range(NCH):
                    if (ot, c) in emitted:
                        continue
                    if c not in chunk_ok and (ot, c) not in sliced:
                        # consider opening early in sliced mode
                        if (not ok(ot, nsp - 1) or n_early >= EARLY_CAP
                                or not any((c, s, 0) in xh_ok
                                           for s in range(subtiles(c)))):
                            continue
                        sliced.add((ot, c))
                        n_early += 1
                    if (ot, c) not in unit_pacc:
                        unit_pacc[(ot, c)] = pacc_pool.tile(
                            [128, 512], F32, name="pacc", tag="pacc"
                        )
                    pacc = unit_pacc[(ot, c)]
                    if (ot, c) in sliced:
                        done = True
                        for s in range(subtiles(c)):
                            si = slice_si.get((ot, c, s), 0)
                            while (si < nsp and ok(ot, si)
                                   and (c, s, w_spans(ot)[si][0] // KH)
                                   in xh_ok):
                                mm_span(ot, c, pacc, si, s=s)
                                si += 1
                                slice_si[(ot, c, s)] = si
                            done = done and si == nsp
                    else:
                        if c not in chunk_ok:
                            continue
                        si = unit_si.get((ot, c), 0)
                        while si < nsp and ok(ot, si):
                            mm_span(ot, c, pacc, si)
                            si += 1
                            unit_si[(ot, c)] = si
                        done = si == nsp
                    if done:
                        finish_unit(ot, c, unit_pacc.pop((ot, c)))
                        emitted.add((ot, c))

        for item in order:
            if item[0] == "w":
                wseq[(item[1], item[2])] = g
                g += 1

        # ---- emission: preps in bus order; matmuls lag W prep by MM_LAG
        # spans and are pumped before each prep. Units of a still-loading
        # chunk may open early in sliced mode (per 128-col sub-tile) so PE
        # has work during chunk loads; at most EARLY_CAP such units stay
        # open (each holds a pacc bank until its chunk completes). ----
        EARLY_CAP = 12
        x_done = {}          # c -> count of prepped halves
        xh_ok = set()        # (c, s, h) prepped
        chunk_ok = set()     # chunks fully prepped
        prepped_g = -1       # highest prepped W span gid (contiguous)
        unit_si = {}         # (ot, c) -> next span index (whole mode)
        slice_done = set()   # (ot, c, s) sliced sub-tiles finished
        sliced = set()       # units opened in sliced mode
        unit_pacc = {}       # (ot, c) -> open pacc tile (whole mode)
        unit_osb = {}        # (ot, c) -> open osb tile (sliced mode)
        emitted = set()      # finished units

        def subtiles(c):
            return CHUNKS[c][1] // 128

        def pump(lag=MM_LAG, final=False):
            limit = prepped_g if final else prepped_g - lag

            def ok(ot, si):
                return wseq[(ot, si)] <= limit

            n_early = len(unit_osb)
            for ot in range(OT):
                if not ok(ot, 0):
                    continue
                nsp = len(w_spans(ot))
                for c in range(NCH):
                    if (ot, c) in emitted:
                        continue
                    if c not in chunk_ok and (ot, c) not in sliced:
                        # consider opening early in sliced mode
                        if (not ok(ot, nsp - 1) or n_early >= EARLY_CAP
                                or not any((c, s, 0) in xh_ok
                                           and (c, s, 1) in xh_ok
                                           for s in range(subtiles(c)))):
                            continue
                        sliced.add((ot, c))
                        unit_osb[(ot, c)] = outsb.tile(
                            [128, CHUNKS[c][1]], F16, name="osb", tag="osb"
                        )
                        n_early += 1
                    if (ot, c) in sliced:
                        # per-sub-tile full-k slices: short-lived pacc bank,
                        # bias-added straight into the unit's osb slice
                        if not ok(ot, nsp - 1):
                            continue
                        osb = unit_osb[(ot, c)]
                        done = True
                        for s in range(subtiles(c)):
                            if (ot, c, s) in slice_done:
                                continue
                            if ((c, s, 0) not in xh_ok
                                    or (c, s, 1) not in xh_ok):
                                done = False
                                continue
                            ps = pacc_pool.tile(
                                [128, 512], F32, name="pacc", tag="pacc"
                            )
                            for si in range(nsp):
                                mm_span(ot, c, ps, si, s=s)
                            nc.vector.tensor_scalar(
                                osb[:, s * 128:(s + 1) * 128],
                                ps[:, s * 128:(s + 1) * 128],
                                bias_sb[:, ot:ot + 1], None, ALU.add,
                            )
                            slice_done.add((ot, c, s))
                        if done:
                            b0, wc = CHUNKS[c]
                            nc.gpsimd.dma_start(
                                out[ot * 128:(ot + 1) * 128, b0:b0 + wc],
                                osb[:],
                            )
                            unit_osb.pop((ot, c))
                            emitted.add((ot, c))
                    else:
                        if c not in chunk_ok:
                            continue
                        if (ot, c) not in unit_pacc:
                            unit_pacc[(ot, c)] = pacc_pool.tile(
                                [128, 512], F32, name="pacc", tag="pacc"
                            )
                        pacc = unit_pacc[(ot, c)]
                        si = unit_si.get((ot, c), 0)
                        while si < nsp and ok(ot, si):
                            mm_span(ot, c, pacc, si)
                            si += 1
                            unit_si[(ot, c)] = si
                        if si == nsp:
                            finish_unit(ot, c, unit_pacc.pop((ot, c)))
                            emitted.add((ot, c))

        for item in order:
            if item[0] == "w":
                wseq[(item[1], item[2])] = g
                g += 1

        # ---- emission: preps in bus order; matmuls lag W prep by MM_LAG
        # spans and are pumped before each prep. Units of a still-loading
        # chunk may open early in sliced mode (per 128-col sub-tile) so PE
        # has work during chunk loads; at most EARLY_CAP such units stay
        # open (each holds a pacc bank until its chunk completes). ----
        EARLY_CAP = 6
        x_done = {}          # c -> count of prepped halves
        xh_ok = set()        # (c, s, h) prepped
        chunk_ok = set()     # chunks fully prepped
        prepped_g = -1       # highest prepped W span gid (contiguous)
        unit_si = {}         # (ot, c) -> next span index (whole mode)
        slice_si = {}        # (ot, c, s) -> next span index (sliced mode)
        sliced = set()       # units opened in sliced mode
        unit_pacc = {}       # (ot, c) -> open pacc tile
        emitted = set()      # finished units

        def subtiles(c):
            return CHUNKS[c][1] // 128

        def pump(lag=MM_LAG, final=False):
            limit = prepped_g if final else prepped_g - lag

            def ok(ot, si):
                return wseq[(ot, si)] <= limit

            n_early = len([u for u in unit_pacc if u in sliced])
            for ot in range(OT):
                if not ok(ot, 0):
                    continue
                nsp = len(w_spans(ot))
                for c in range(NCH):
                    if (ot, c) in emitted:
                        continue
                    if c not in chunk_ok and (ot, c) not in sliced:
                        # consider opening early in sliced mode
                        if (not ok(ot, nsp - 1) or n_early >= EARLY_CAP
                                or not any((c, s, 0) in xh_ok
                                           for s in range(subtiles(c)))):
                            continue
                        sliced.add((ot, c))
                        n_early += 1
                    if (ot, c) not in unit_pacc:
                        unit_pacc[(ot, c)] = pacc_pool.tile(
                            [128, 512], F32, name="pacc", tag="pacc"
                        )
                    pacc = unit_pacc[(ot, c)]
                    if (ot, c) in sliced:
                        done = True
                        for s in range(subtiles(c)):
                            si = slice_si.get((ot, c, s), 0)
                            while (si < nsp and ok(ot, si)
                                   and (c, s, w_spans(ot)[si][0] // KH)
                                   in xh_ok):
                                mm_span(ot, c, pacc, si, s=s)
                                si += 1
                                slice_si[(ot, c, s)] = si
                            done = done and si == nsp
                    else:
                        if c not in chunk_ok:
                            continue
                        si = unit_si.get((ot, c), 0)
                        while si < nsp and ok(ot, si):
                            mm_span(ot, c, pacc, si)
                            si += 1
                            unit_si[(ot, c)] = si
                        done = si == nsp
                    if done:
                        finish_unit(ot, c, unit_pacc.pop((ot, c)))
                        emitted.add((ot, c))

        for item in order:
            if item[0] == "x":
                pump(lag=0)
                _, c, s, h = item
                prep_x_half(c, s, h)
                x_done[c] = x_done.get(c, 0) + 1
                xh_ok.add((c, s, h))
                if x_done[c] == len(chunk_items(c)):
                    chunk_ok.add(c)
            else:
                pump()
                _, ot, si = item
                prep_w_span(ot, si)
                prepped_g = max(prepped_g, wseq[(ot, si)])
        pump(final=True)
        assert len(emitted) == OT * NCH, f"only {len(emitted)} units emitted"

    nc.compile()
    return nc


_NC_CACHE = None


def kernel(x: np.ndarray, weight: np.ndarray, bias: np.ndarray) -> np.ndarray:
    global _NC_CACHE
    if _NC_CACHE is None:
        _NC_CACHE = _build()
    nc = _NC_CACHE

    x = np.ascontiguousarray(np.asarray(x, dtype=np.float32))
    weight = np.ascontiguousarray(np.asarray(weight, dtype=np.float32))
    bias = np.ascontiguousarray(np.asarray(bias, dtype=np.float32))

    in_maps = []
    for c in range(N_CORES):
        bi, oi = c // P_O, c % P_O
        in_maps.append(
            {
                "x": x[bi * Bs:(bi + 1) * Bs],
                "w": weight[oi * Os:(oi + 1) * Os],
                "b": bias[oi * Os:(oi + 1) * Os],
            }
        )

    res = run_bass_kernel_spmd(nc, in_maps, list(range(N_CORES)))

    out = np.empty((B, OUT), dtype=np.float32)
    for c in range(N_CORES):
        bi, oi = c // P_O, c % P_O
        out[bi * Bs:(bi + 1) * Bs, oi * Os:(oi + 1) * Os] = (
            res.results[c]["out"].T.astype(np.float32)
        )
    return out
